# revision 1
# baseline (speedup 1.0000x reference)
"""Trainium2 Bass kernel for fixed-degree SpMM (gather + weighted reduce).

out[n, b] = sum_k values[n, k] * x[indices[n, k], b] + bias[n]

N=1M rows, K=32 nnz/row, B=8 dense cols. Sharded row-wise across 8
NeuronCores; x (32MB) is replicated to every core's HBM and gathered
with GPSIMD indirect DMA (one 32B descriptor per (n,k)). The weighted
K-reduction runs on the vector engine as an in-place binary tree over
contiguous halves.
"""
import sys

if "/opt/trn_rl_repo" not in sys.path:
    sys.path.insert(0, "/opt/trn_rl_repo")

from contextlib import ExitStack

import numpy as np

from concourse import bass, mybir, bacc
import concourse.tile as tile
from concourse.bass_utils import run_bass_kernel_spmd

# Problem constants (hardcoded per harness contract)
N = 1_000_000
K = 32
B = 8
NCORES = 8

P = 128                 # SBUF partitions
RPP = 977               # rows per partition per core (padded)
ROWS_CORE = P * RPP     # 125_056
N_PAD = ROWS_CORE * NCORES  # 1_000_448
CH = 16                 # chunk: rows per partition processed per iteration

F32 = mybir.dt.float32
I32 = mybir.dt.int32

_compiled = None


def _build():
    nc = bacc.Bacc(
        "TRN2",
        target_bir_lowering=False,
        debug=False,
        enable_asserts=False,
        num_devices=NCORES,
    )
    idx_d = nc.dram_tensor("idx", [P, RPP * K], I32, kind="ExternalInput").ap()
    val_d = nc.dram_tensor("val", [P, RPP * K], F32, kind="ExternalInput").ap()
    bias_d = nc.dram_tensor("bias", [P, RPP], F32, kind="ExternalInput").ap()
    x_d = nc.dram_tensor("x", [N, B], F32, kind="ExternalInput").ap()
    out_d = nc.dram_tensor("out", [P, RPP * B], F32, kind="ExternalOutput").ap()

    with tile.TileContext(nc) as tc, ExitStack() as ctx:
        io_pool = ctx.enter_context(tc.tile_pool(name="io", bufs=2))
        g_pool = ctx.enter_context(tc.tile_pool(name="g", bufs=2))
        acc_pool = ctx.enter_context(tc.tile_pool(name="acc", bufs=1))

        out_sb = acc_pool.tile([P, RPP * B], F32)
        bias_sb = acc_pool.tile([P, RPP], F32)
        nc.sync.dma_start(bias_sb[:], bias_d[:])

        c0 = 0
        while c0 < RPP:
            cw = min(CH, RPP - c0)
            idx_sb = io_pool.tile([P, CH * K], I32, tag="idx")
            val_sb = io_pool.tile([P, CH * K], F32, tag="val")
            nc.sync.dma_start(idx_sb[:, : cw * K], idx_d[:, c0 * K : (c0 + cw) * K])
            nc.sync.dma_start(val_sb[:, : cw * K], val_d[:, c0 * K : (c0 + cw) * K])

            g = g_pool.tile([P, CH * K * B], F32, tag="g")
            # one indirect DMA per (row-in-chunk, k): 128 gathered rows each
            # (HW limit: one offset per partition per indirect instruction)
            for m in range(cw * K):
                nc.gpsimd.indirect_dma_start(
                    out=g[:, m * B : (m + 1) * B],
                    out_offset=None,
                    in_=x_d[:],
                    in_offset=bass.IndirectOffsetOnAxis(
                        ap=idx_sb[:, m : m + 1], axis=0
                    ),
                )

            # multiply by values (broadcast over B), in place
            gv = g[:, : cw * K * B].rearrange("p (m b) -> p m b", b=B)
            vv = val_sb[:, : cw * K].unsqueeze(2).broadcast_to([P, cw * K, B])
            nc.vector.tensor_tensor(out=gv, in0=gv, in1=vv, op=mybir.AluOpType.mult)

            # tree-reduce over k (in place, contiguous halves)
            g4 = g[:, : cw * K * B].rearrange("p (i k b) -> p i k b", k=K, b=B)
            kk = K
            while kk > 1:
                h = kk // 2
                lo = g4[:, :, 0:h, :]
                hi = g4[:, :, h:kk, :]
                nc.vector.tensor_tensor(out=lo, in0=lo, in1=hi, op=mybir.AluOpType.add)
                kk = h

            # add bias, write into the output accumulator
            red = g4[:, :, 0:1, :].squeeze(2)
            biasv = bias_sb[:, c0 : c0 + cw].unsqueeze(2).broadcast_to([P, cw, B])
            outv = out_sb[:, c0 * B : (c0 + cw) * B].rearrange("p (i b) -> p i b", b=B)
            nc.vector.tensor_tensor(out=outv, in0=red, in1=biasv, op=mybir.AluOpType.add)
            c0 += cw

        nc.sync.dma_start(out_d[:], out_sb[:])

    nc.compile()
    return nc


def _get_nc():
    global _compiled
    if _compiled is None:
        _compiled = _build()
    return _compiled


def _shard_inputs(x, values, bias, indices):
    pad = N_PAD - N
    idx_p = np.concatenate([indices, np.zeros((pad, K), np.int32)])
    val_p = np.concatenate([values, np.zeros((pad, K), np.float32)])
    bias_p = np.concatenate([bias, np.zeros((pad,), np.float32)])
    x = np.ascontiguousarray(x, dtype=np.float32)
    in_maps = []
    for c in range(NCORES):
        s = slice(c * ROWS_CORE, (c + 1) * ROWS_CORE)
        in_maps.append(
            {
                "idx": np.ascontiguousarray(idx_p[s].reshape(P, RPP * K)),
                "val": np.ascontiguousarray(val_p[s].reshape(P, RPP * K)),
                "bias": np.ascontiguousarray(bias_p[s].reshape(P, RPP)),
                "x": x,
            }
        )
    return in_maps


def run(x, values, bias, indices, trace=False, **spmd_kwargs):
    nc = _get_nc()
    in_maps = _shard_inputs(x, values, bias, indices)
    res = run_bass_kernel_spmd(
        nc, in_maps, core_ids=list(range(NCORES)), trace=trace, **spmd_kwargs
    )
    shards = [r["out"].reshape(ROWS_CORE, B) for r in res.results]
    out = np.concatenate(shards)[:N]
    return out, res


def kernel(x, values, bias, indices):
    out, _ = run(
        np.asarray(x), np.asarray(values), np.asarray(bias), np.asarray(indices)
    )
    return out

